# revision 34
# baseline (speedup 1.0000x reference)
"""Trainium2 Bass kernel for nn_Euclid_FC: out[b,o] = -0.5 * ||x[b,:] - W[:,o]||^2.

Computed as x@W - 0.5*||x_b||^2 - 0.5*||w_o||^2, i.e. a 2048x1024x4096
GEMM plus rank-1 bias terms.

Sharding (8 cores): 2-way over batch x 4-way over the output dim. Each core
computes a [1024, 1024] output block from x^T slice [1024, 1024] and W slice
[1024, 1024] (the traffic-minimal split: 8.5 MiB HBM traffic per core).

Device kernel structure per core:
  - inputs land in SBUF via a few large pi-major-layout DMAs (host
    pre-transposes x and pre-packs [partition, K-subtile, free] order);
  - the GEMM runs as fp8-e4m3 DoubleRow matmuls (2 K-subtiles per matmul,
    ~2x bf16 throughput), accumulating fp32 in PSUM;
  - the rank-1 terms -0.5||x_b||^2 and -0.5||w_o||^2 are precomputed on the
    host (exact fp32), combined into per-tile bias tiles on the otherwise
    idle Scalar engine, and added in the single mandatory PSUM->SBUF pass
    on the Vector engine;
  - dummy warmup matmuls run during the DMA head so the PE clock gate (HAM)
    is open when real matmuls begin;
  - the output is written as fp16 row bands (halves writeback traffic) and
    upcast to fp32 on the host after the gather.

Variants kept for fallback: bfloat16 (rel err ~7e-5) and float32r (fp32 with
11 mantissa bits, rel err ~5e-6) at ~47-65us vs ~34us for fp8.
Measured end-to-end: HW exec ~33.9us, norm rel err ~1.2e-3.
"""

import sys

if "/opt/trn_rl_repo" not in sys.path:
    sys.path.insert(0, "/opt/trn_rl_repo")

import ml_dtypes
import numpy as np

BATCH, D_IN, D_OUT = 2048, 1024, 4096
N_CORES = 8
R, C = 2, 4  # batch split x out-dim split
BB = BATCH // R  # 1024 batch rows per core
OO = D_OUT // C  # 1024 out cols per core
KT = D_IN // 128  # 8 real K-tiles
KA = KT + 1  # +1 augmented bias K-tile
P = 128

_cached = {}


def _round_fp32r(a):
    """Round fp32 array to fp32r (11 explicit mantissa bits), RTNE."""
    b = np.ascontiguousarray(a, dtype=np.float32).view(np.uint32).copy()
    bias = ((b >> 12) & 1) + 0x7FF
    b += bias
    b &= np.uint32(0xFFFFF000)
    return b.view(np.float32)


def _build_program(mm_dtype_name="float32r", out16=False):
    import concourse.mybir as mybir
    import concourse.tile as tile
    from concourse import bacc

    f32 = mybir.dt.float32
    mm_dt = getattr(mybir.dt, mm_dtype_name)

    nc = bacc.Bacc("TRN2", target_bir_lowering=False, debug=False, num_devices=N_CORES)
    # pi-major layout: [partition, K-subtile, free] so chunk DMAs read long
    # contiguous runs per partition
    xt_d = nc.dram_tensor("xt", [P, KT, BB], mm_dt, kind="ExternalInput").ap()
    w_d = nc.dram_tensor("w", [P, KT, OO], mm_dt, kind="ExternalInput").ap()
    # wsq replicated across partitions; xsq laid out [b % 128, b // 128]
    wsq_d = nc.dram_tensor("wsq", [P, OO], f32, kind="ExternalInput").ap()
    xsq_d = nc.dram_tensor("xsq", [P, BB // P], f32, kind="ExternalInput").ap()
    out_dt = mybir.dt.float16 if out16 else f32
    out_d = nc.dram_tensor("out", [BB, OO], out_dt, kind="ExternalOutput").ap()

    ident = mybir.ActivationFunctionType.Identity
    add = mybir.AluOpType.add

    M_TILES = BB // P  # 8
    N_TILES = OO // 512  # 2

    with tile.TileContext(nc) as tc:
        with (
            tc.tile_pool(name="ops", bufs=1) as opool,
            tc.tile_pool(name="outp", bufs=4) as outpool,
            tc.tile_pool(name="otp", bufs=8) as otpool,
            tc.tile_pool(name="ps", bufs=8, space="PSUM") as pspool,
        ):
            # DMA issue costs ~620ns of sequencer time per dma_start, so spread
            # the issues across four engines' DMA queues to parallelize.
            dma_engines = [nc.sync, nc.scalar]

            fp8 = mm_dtype_name == "float8e4"
            # matmul contraction granularity (2 K-subtiles for fp8 DoubleRow)
            mm_ksub = 2 if fp8 else 1
            # DMA chunk sizes in K-subtiles: a small first chunk lets the first
            # matmul group start early; coarser later chunks amortize the
            # ~600ns per-dma_start sequencer issue cost.
            chunk_sizes = [4, 4] if fp8 else [1] * KT
            assert sum(chunk_sizes) == KT
            chunk_starts = [sum(chunk_sizes[:i]) for i in range(len(chunk_sizes))]
            n_dma = len(chunk_sizes)

            xt_sb = [None] * n_dma
            w_sb = [None] * n_dma
            for k in range(n_dma):
                c0, cs = chunk_starts[k], chunk_sizes[k]
                xtk = opool.tile([P, cs, BB], mm_dt, tag=f"xt{k}")
                nc.sync.dma_start(xtk[:], xt_d[:, c0 : c0 + cs, :])
                wk = opool.tile([P, cs, OO], mm_dt, tag=f"w{k}")
                nc.sync.dma_start(wk[:], w_d[:, c0 : c0 + cs, :])
                xt_sb[k] = xtk
                w_sb[k] = wk
                if k == n_dma - 1:
                    wsq_sb = opool.tile([P, OO], f32, tag="wsq")
                    nc.scalar.dma_start(wsq_sb[:], wsq_d[:])
                    xsq_sb = opool.tile([P, BB // P], f32, tag="xsq")
                    nc.scalar.dma_start(xsq_sb[:], xsq_d[:])

            # map mm index -> (dma chunk, subtile offset within chunk)
            mm_map = []
            for k in range(n_dma):
                for s in range(0, chunk_sizes[k], mm_ksub):
                    mm_map.append((k, s))
            n_mm = len(mm_map)

            # PE warmup: dummy matmuls on a zeroed tile while DMAs stream, so
            # the HAM clock gate opens before the real matmuls begin.
            warm_in = opool.tile([P, 512], mm_dt, tag="warm")
            nc.vector.memset(warm_in[:], 0)
            warm_ps = pspool.tile([P, 512], f32, tag="ps")
            for _ in range(16):
                nc.tensor.matmul(
                    warm_ps[:],
                    lhsT=warm_in[:, :P],
                    rhs=warm_in[:],
                    start=True,
                    stop=True,
                )

            # combined bias tiles on ScalarE (idle during the DMA head):
            # bias_mn[b, o] = xsq[b] + wsq[o]
            bias_sb = []
            for m in range(M_TILES):
                for n in range(N_TILES):
                    bt = outpool.tile([P, 512], f32, tag=f"bias{m}_{n}")
                    nc.scalar.activation(
                        out=bt[:],
                        in_=wsq_sb[:, n * 512 : (n + 1) * 512],
                        func=ident,
                        bias=xsq_sb[:, m : m + 1],
                    )
                    bias_sb.append(bt)

            perf_mode = mybir.MatmulPerfMode.DoubleRow if fp8 else None
            for m in range(M_TILES):
                # full [128, OO] output band staged in SBUF, written by one DMA
                ot = otpool.tile([P, OO], out_dt, tag="ot")
                for n in range(N_TILES):
                    ps = pspool.tile([P, 512], f32, tag="ps")
                    for k in range(n_mm):
                        kd, ki = mm_map[k]
                        if fp8:
                            lhsT = xt_sb[kd][
                                :, ki : ki + 2, m * P : (m + 1) * P
                            ]
                            rhs = w_sb[kd][
                                :, ki : ki + 2, n * 512 : (n + 1) * 512
                            ]
                        else:
                            lhsT = xt_sb[kd][:, ki, m * P : (m + 1) * P]
                            rhs = w_sb[kd][:, ki, n * 512 : (n + 1) * 512]
                        nc.tensor.matmul(
                            ps[:],
                            lhsT=lhsT,
                            rhs=rhs,
                            start=(k == 0),
                            stop=(k == n_mm - 1),
                            perf_mode=perf_mode,
                        )
                    nc.vector.tensor_tensor(
                        ot[:, n * 512 : (n + 1) * 512],
                        ps[:],
                        bias_sb[m * N_TILES + n][:],
                        add,
                    )
                if m == M_TILES - 1:
                    # split the last band across both sequencers so the final
                    # writeback drains in half the time
                    nc.sync.dma_start(
                        out_d[m * P : (m + 1) * P, : OO // 2], ot[:, : OO // 2]
                    )
                    nc.scalar.dma_start(
                        out_d[m * P : (m + 1) * P, OO // 2 :], ot[:, OO // 2 :]
                    )
                else:
                    dma_engines[m % 2].dma_start(out_d[m * P : (m + 1) * P, :], ot[:])
    nc.compile()
    return nc


def _to_mm(a, mm_dtype_name):
    if mm_dtype_name == "bfloat16":
        return a.astype(ml_dtypes.bfloat16)
    if mm_dtype_name == "float8e4":
        return a.astype(ml_dtypes.float8_e4m3)
    if mm_dtype_name == "float32r":
        return _round_fp32r(a)
    return a.astype(np.float32)


def _hi_lo(v, mm_dtype_name):
    """Split fp64 vector into hi+lo parts representable in the mm dtype."""
    hi = _to_mm(v.astype(np.float32), mm_dtype_name)
    lo = _to_mm((v - hi.astype(np.float64)).astype(np.float32), mm_dtype_name)
    return hi, lo


def _shard_inputs(x, W, mm_dtype_name):
    """Build per-core in_maps: augmented x^T and W slices."""
    x = np.asarray(x, dtype=np.float32)
    W = np.asarray(W, dtype=np.float32)
    xsqh = -0.5 * np.einsum("bi,bi->b", x.astype(np.float64), x.astype(np.float64))
    wsqh = -0.5 * np.einsum("io,io->o", W.astype(np.float64), W.astype(np.float64))

    def pi_major(a2d, free):
        """[K, free] -> [P, KT, free] (partition-major)."""
        return np.ascontiguousarray(
            a2d.reshape(KT, P, free).transpose(1, 0, 2)
        )

    xt_shards = []
    xsq_shards = []
    for i in range(R):
        xs = x[i * BB : (i + 1) * BB]
        xt_shards.append(pi_major(_to_mm(np.ascontiguousarray(xs.T), mm_dtype_name), BB))
        xsq_shards.append(
            np.ascontiguousarray(
                xsqh[i * BB : (i + 1) * BB].astype(np.float32).reshape(BB // P, P).T
            )
        )

    w_shards = []
    wsq_shards = []
    for j in range(C):
        w_shards.append(pi_major(_to_mm(W[:, j * OO : (j + 1) * OO], mm_dtype_name), OO))
        wsq_shards.append(
            np.ascontiguousarray(
                np.broadcast_to(
                    wsqh[j * OO : (j + 1) * OO].astype(np.float32), (P, OO)
                )
            )
        )

    in_maps = []
    for core in range(N_CORES):
        i, j = divmod(core, C)
        in_maps.append(
            {
                "xt": xt_shards[i],
                "w": w_shards[j],
                "xsq": xsq_shards[i],
                "wsq": wsq_shards[j],
            }
        )
    return in_maps


def _gather(results):
    out = np.empty((BATCH, D_OUT), dtype=np.float32)
    for core in range(N_CORES):
        i, j = divmod(core, C)
        out[i * BB : (i + 1) * BB, j * OO : (j + 1) * OO] = results[core][
            "out"
        ].astype(np.float32)
    return out


def run(x, W, trace=False, mm_dtype_name="float32r", out16=False):
    from concourse import bass_utils

    key = (mm_dtype_name, out16)
    if key not in _cached:
        _cached[key] = _build_program(mm_dtype_name, out16)
    nc = _cached[key]
    in_maps = _shard_inputs(x, W, mm_dtype_name)
    res = bass_utils.run_bass_kernel_spmd(
        nc, in_maps, core_ids=list(range(N_CORES)), trace=trace
    )
    return _gather(res.results), res


def kernel(x, W):
    out, _ = run(x, W, trace=False, mm_dtype_name="float8e4", out16=True)
    return out


# revision 35
# speedup vs baseline: 1.0101x; 1.0101x over previous
"""Trainium2 Bass kernel for nn_Euclid_FC: out[b,o] = -0.5 * ||x[b,:] - W[:,o]||^2.

Computed as x@W - 0.5*||x_b||^2 - 0.5*||w_o||^2, i.e. a 2048x1024x4096
GEMM plus rank-1 bias terms.

Sharding (8 cores): 2-way over batch x 4-way over the output dim. Each core
computes a [1024, 1024] output block from x^T slice [1024, 1024] and W slice
[1024, 1024] (the traffic-minimal split: 8.5 MiB HBM traffic per core).

Device kernel structure per core:
  - inputs land in SBUF via a few large pi-major-layout DMAs (host
    pre-transposes x and pre-packs [partition, K-subtile, free] order);
  - the GEMM runs as fp8-e4m3 DoubleRow matmuls (2 K-subtiles per matmul,
    ~2x bf16 throughput), accumulating fp32 in PSUM;
  - the rank-1 terms -0.5||x_b||^2 and -0.5||w_o||^2 are precomputed on the
    host (exact fp32), combined into per-tile bias tiles on the otherwise
    idle Scalar engine, and added in the single mandatory PSUM->SBUF pass
    on the Vector engine;
  - dummy warmup matmuls run during the DMA head so the PE clock gate (HAM)
    is open when real matmuls begin;
  - the output is written as fp16 row bands (halves writeback traffic) and
    upcast to fp32 on the host after the gather.

Variants kept for fallback: bfloat16 (rel err ~7e-5) and float32r (fp32 with
11 mantissa bits, rel err ~5e-6) at ~47-65us vs ~34us for fp8.
Measured end-to-end: HW exec ~33.9us, norm rel err ~1.2e-3.
"""

import sys

if "/opt/trn_rl_repo" not in sys.path:
    sys.path.insert(0, "/opt/trn_rl_repo")

import ml_dtypes
import numpy as np

BATCH, D_IN, D_OUT = 2048, 1024, 4096
N_CORES = 8
R, C = 2, 4  # batch split x out-dim split
BB = BATCH // R  # 1024 batch rows per core
OO = D_OUT // C  # 1024 out cols per core
KT = D_IN // 128  # 8 real K-tiles
KA = KT + 1  # +1 augmented bias K-tile
P = 128

_cached = {}


def _round_fp32r(a):
    """Round fp32 array to fp32r (11 explicit mantissa bits), RTNE."""
    b = np.ascontiguousarray(a, dtype=np.float32).view(np.uint32).copy()
    bias = ((b >> 12) & 1) + 0x7FF
    b += bias
    b &= np.uint32(0xFFFFF000)
    return b.view(np.float32)


def _build_program(mm_dtype_name="float32r", out16=False):
    import concourse.mybir as mybir
    import concourse.tile as tile
    from concourse import bacc

    f32 = mybir.dt.float32
    mm_dt = getattr(mybir.dt, mm_dtype_name)

    nc = bacc.Bacc("TRN2", target_bir_lowering=False, debug=False, num_devices=N_CORES)
    # pi-major layout: [partition, K-subtile, free] so chunk DMAs read long
    # contiguous runs per partition
    xt_d = nc.dram_tensor("xt", [P, KT, BB], mm_dt, kind="ExternalInput").ap()
    w_d = nc.dram_tensor("w", [P, KT, OO], mm_dt, kind="ExternalInput").ap()
    # wsq replicated across partitions; xsq laid out [b % 128, b // 128]
    wsq_d = nc.dram_tensor("wsq", [P, OO], f32, kind="ExternalInput").ap()
    xsq_d = nc.dram_tensor("xsq", [P, BB // P], f32, kind="ExternalInput").ap()
    out_dt = mybir.dt.float16 if out16 else f32
    out_d = nc.dram_tensor("out", [BB, OO], out_dt, kind="ExternalOutput").ap()

    ident = mybir.ActivationFunctionType.Identity
    add = mybir.AluOpType.add

    M_TILES = BB // P  # 8
    N_TILES = OO // 512  # 2

    with tile.TileContext(nc) as tc:
        with (
            tc.tile_pool(name="ops", bufs=1) as opool,
            tc.tile_pool(name="outp", bufs=4) as outpool,
            tc.tile_pool(name="otp", bufs=8) as otpool,
            tc.tile_pool(name="ps", bufs=8, space="PSUM") as pspool,
        ):
            # DMA issue costs ~620ns of sequencer time per dma_start, so spread
            # the issues across four engines' DMA queues to parallelize.
            dma_engines = [nc.sync, nc.scalar]

            fp8 = mm_dtype_name == "float8e4"
            # matmul contraction granularity (2 K-subtiles for fp8 DoubleRow)
            mm_ksub = 2 if fp8 else 1
            # DMA chunk sizes in K-subtiles: a small first chunk lets the first
            # matmul group start early; coarser later chunks amortize the
            # ~600ns per-dma_start sequencer issue cost.
            chunk_sizes = [4, 2, 2] if fp8 else [1] * KT
            assert sum(chunk_sizes) == KT
            chunk_starts = [sum(chunk_sizes[:i]) for i in range(len(chunk_sizes))]
            n_dma = len(chunk_sizes)

            xt_sb = [None] * n_dma
            w_sb = [None] * n_dma
            for k in range(n_dma):
                c0, cs = chunk_starts[k], chunk_sizes[k]
                xtk = opool.tile([P, cs, BB], mm_dt, tag=f"xt{k}")
                nc.sync.dma_start(xtk[:], xt_d[:, c0 : c0 + cs, :])
                wk = opool.tile([P, cs, OO], mm_dt, tag=f"w{k}")
                nc.sync.dma_start(wk[:], w_d[:, c0 : c0 + cs, :])
                xt_sb[k] = xtk
                w_sb[k] = wk
                if k == n_dma - 1:
                    wsq_sb = opool.tile([P, OO], f32, tag="wsq")
                    nc.scalar.dma_start(wsq_sb[:], wsq_d[:])
                    xsq_sb = opool.tile([P, BB // P], f32, tag="xsq")
                    nc.scalar.dma_start(xsq_sb[:], xsq_d[:])

            # map mm index -> (dma chunk, subtile offset within chunk)
            mm_map = []
            for k in range(n_dma):
                for s in range(0, chunk_sizes[k], mm_ksub):
                    mm_map.append((k, s))
            n_mm = len(mm_map)

            # PE warmup: dummy matmuls on a zeroed tile while DMAs stream, so
            # the HAM clock gate opens before the real matmuls begin.
            warm_in = opool.tile([P, 512], mm_dt, tag="warm")
            nc.vector.memset(warm_in[:], 0)
            warm_ps = pspool.tile([P, 512], f32, tag="ps")
            for _ in range(16):
                nc.tensor.matmul(
                    warm_ps[:],
                    lhsT=warm_in[:, :P],
                    rhs=warm_in[:],
                    start=True,
                    stop=True,
                )

            # combined bias tiles on ScalarE (idle during the DMA head):
            # bias_mn[b, o] = xsq[b] + wsq[o]
            bias_sb = []
            for m in range(M_TILES):
                for n in range(N_TILES):
                    bt = outpool.tile([P, 512], f32, tag=f"bias{m}_{n}")
                    nc.scalar.activation(
                        out=bt[:],
                        in_=wsq_sb[:, n * 512 : (n + 1) * 512],
                        func=ident,
                        bias=xsq_sb[:, m : m + 1],
                    )
                    bias_sb.append(bt)

            perf_mode = mybir.MatmulPerfMode.DoubleRow if fp8 else None
            for m in range(M_TILES):
                # full [128, OO] output band staged in SBUF, written by one DMA
                ot = otpool.tile([P, OO], out_dt, tag="ot")
                for n in range(N_TILES):
                    ps = pspool.tile([P, 512], f32, tag="ps")
                    for k in range(n_mm):
                        kd, ki = mm_map[k]
                        if fp8:
                            lhsT = xt_sb[kd][
                                :, ki : ki + 2, m * P : (m + 1) * P
                            ]
                            rhs = w_sb[kd][
                                :, ki : ki + 2, n * 512 : (n + 1) * 512
                            ]
                        else:
                            lhsT = xt_sb[kd][:, ki, m * P : (m + 1) * P]
                            rhs = w_sb[kd][:, ki, n * 512 : (n + 1) * 512]
                        nc.tensor.matmul(
                            ps[:],
                            lhsT=lhsT,
                            rhs=rhs,
                            start=(k == 0),
                            stop=(k == n_mm - 1),
                            perf_mode=perf_mode,
                        )
                    nc.vector.tensor_tensor(
                        ot[:, n * 512 : (n + 1) * 512],
                        ps[:],
                        bias_sb[m * N_TILES + n][:],
                        add,
                    )
                if m >= M_TILES - 2:
                    # split the last band across both sequencers so the final
                    # writeback drains in half the time
                    nc.sync.dma_start(
                        out_d[m * P : (m + 1) * P, : OO // 2], ot[:, : OO // 2]
                    )
                    nc.scalar.dma_start(
                        out_d[m * P : (m + 1) * P, OO // 2 :], ot[:, OO // 2 :]
                    )
                else:
                    dma_engines[m % 2].dma_start(out_d[m * P : (m + 1) * P, :], ot[:])
    nc.compile()
    return nc


def _to_mm(a, mm_dtype_name):
    if mm_dtype_name == "bfloat16":
        return a.astype(ml_dtypes.bfloat16)
    if mm_dtype_name == "float8e4":
        return a.astype(ml_dtypes.float8_e4m3)
    if mm_dtype_name == "float32r":
        return _round_fp32r(a)
    return a.astype(np.float32)


def _hi_lo(v, mm_dtype_name):
    """Split fp64 vector into hi+lo parts representable in the mm dtype."""
    hi = _to_mm(v.astype(np.float32), mm_dtype_name)
    lo = _to_mm((v - hi.astype(np.float64)).astype(np.float32), mm_dtype_name)
    return hi, lo


def _shard_inputs(x, W, mm_dtype_name):
    """Build per-core in_maps: augmented x^T and W slices."""
    x = np.asarray(x, dtype=np.float32)
    W = np.asarray(W, dtype=np.float32)
    xsqh = -0.5 * np.einsum("bi,bi->b", x.astype(np.float64), x.astype(np.float64))
    wsqh = -0.5 * np.einsum("io,io->o", W.astype(np.float64), W.astype(np.float64))

    def pi_major(a2d, free):
        """[K, free] -> [P, KT, free] (partition-major)."""
        return np.ascontiguousarray(
            a2d.reshape(KT, P, free).transpose(1, 0, 2)
        )

    xt_shards = []
    xsq_shards = []
    for i in range(R):
        xs = x[i * BB : (i + 1) * BB]
        xt_shards.append(pi_major(_to_mm(np.ascontiguousarray(xs.T), mm_dtype_name), BB))
        xsq_shards.append(
            np.ascontiguousarray(
                xsqh[i * BB : (i + 1) * BB].astype(np.float32).reshape(BB // P, P).T
            )
        )

    w_shards = []
    wsq_shards = []
    for j in range(C):
        w_shards.append(pi_major(_to_mm(W[:, j * OO : (j + 1) * OO], mm_dtype_name), OO))
        wsq_shards.append(
            np.ascontiguousarray(
                np.broadcast_to(
                    wsqh[j * OO : (j + 1) * OO].astype(np.float32), (P, OO)
                )
            )
        )

    in_maps = []
    for core in range(N_CORES):
        i, j = divmod(core, C)
        in_maps.append(
            {
                "xt": xt_shards[i],
                "w": w_shards[j],
                "xsq": xsq_shards[i],
                "wsq": wsq_shards[j],
            }
        )
    return in_maps


def _gather(results):
    out = np.empty((BATCH, D_OUT), dtype=np.float32)
    for core in range(N_CORES):
        i, j = divmod(core, C)
        out[i * BB : (i + 1) * BB, j * OO : (j + 1) * OO] = results[core][
            "out"
        ].astype(np.float32)
    return out


def run(x, W, trace=False, mm_dtype_name="float32r", out16=False):
    from concourse import bass_utils

    key = (mm_dtype_name, out16)
    if key not in _cached:
        _cached[key] = _build_program(mm_dtype_name, out16)
    nc = _cached[key]
    in_maps = _shard_inputs(x, W, mm_dtype_name)
    res = bass_utils.run_bass_kernel_spmd(
        nc, in_maps, core_ids=list(range(N_CORES)), trace=trace
    )
    return _gather(res.results), res


def kernel(x, W):
    out, _ = run(x, W, trace=False, mm_dtype_name="float8e4", out16=True)
    return out


# revision 36
# speedup vs baseline: 1.0338x; 1.0234x over previous
"""Trainium2 Bass kernel for nn_Euclid_FC: out[b,o] = -0.5 * ||x[b,:] - W[:,o]||^2.

Computed as x@W - 0.5*||x_b||^2 - 0.5*||w_o||^2, i.e. a 2048x1024x4096
GEMM plus rank-1 bias terms.

Sharding (8 cores): 2-way over batch x 4-way over the output dim. Each core
computes a [1024, 1024] output block from x^T slice [1024, 1024] and W slice
[1024, 1024] (the traffic-minimal split: 8.5 MiB HBM traffic per core).

Device kernel structure per core:
  - inputs land in SBUF via a few large pi-major-layout DMAs (host
    pre-transposes x and pre-packs [partition, K-subtile, free] order);
  - the GEMM runs as fp8-e4m3 DoubleRow matmuls (2 K-subtiles per matmul,
    ~2x bf16 throughput), accumulating fp32 in PSUM;
  - the rank-1 terms -0.5||x_b||^2 and -0.5||w_o||^2 are precomputed on the
    host (exact fp32), combined into per-tile bias tiles on the otherwise
    idle Scalar engine, and added in the single mandatory PSUM->SBUF pass
    on the Vector engine;
  - dummy warmup matmuls run during the DMA head so the PE clock gate (HAM)
    is open when real matmuls begin;
  - the output is written as fp16 row bands (halves writeback traffic) and
    upcast to fp32 on the host after the gather.

Variants kept for fallback: bfloat16 (rel err ~7e-5) and float32r (fp32 with
11 mantissa bits, rel err ~5e-6) at ~47-65us vs ~34us for fp8.
Measured end-to-end: HW exec ~33.9us, norm rel err ~1.2e-3.
"""

import sys

if "/opt/trn_rl_repo" not in sys.path:
    sys.path.insert(0, "/opt/trn_rl_repo")

import ml_dtypes
import numpy as np

BATCH, D_IN, D_OUT = 2048, 1024, 4096
N_CORES = 8
R, C = 2, 4  # batch split x out-dim split
BB = BATCH // R  # 1024 batch rows per core
OO = D_OUT // C  # 1024 out cols per core
KT = D_IN // 128  # 8 real K-tiles
KA = KT + 1  # +1 augmented bias K-tile
P = 128

_cached = {}


def _round_fp32r(a):
    """Round fp32 array to fp32r (11 explicit mantissa bits), RTNE."""
    b = np.ascontiguousarray(a, dtype=np.float32).view(np.uint32).copy()
    bias = ((b >> 12) & 1) + 0x7FF
    b += bias
    b &= np.uint32(0xFFFFF000)
    return b.view(np.float32)


def _build_program(mm_dtype_name="float32r", out16=False):
    import concourse.mybir as mybir
    import concourse.tile as tile
    from concourse import bacc

    f32 = mybir.dt.float32
    mm_dt = getattr(mybir.dt, mm_dtype_name)

    nc = bacc.Bacc("TRN2", target_bir_lowering=False, debug=False, num_devices=N_CORES)
    # pi-major layout: [partition, K-subtile, free] so chunk DMAs read long
    # contiguous runs per partition
    xt_d = nc.dram_tensor("xt", [P, KT, BB], mm_dt, kind="ExternalInput").ap()
    w_d = nc.dram_tensor("w", [P, KT, OO], mm_dt, kind="ExternalInput").ap()
    # wsq replicated across partitions; xsq laid out [b % 128, b // 128]
    wsq_d = nc.dram_tensor("wsq", [P, OO], f32, kind="ExternalInput").ap()
    xsq_d = nc.dram_tensor("xsq", [P, BB // P], f32, kind="ExternalInput").ap()
    out_dt = mybir.dt.float16 if out16 else f32
    out_d = nc.dram_tensor("out", [BB, OO], out_dt, kind="ExternalOutput").ap()

    ident = mybir.ActivationFunctionType.Identity
    add = mybir.AluOpType.add

    M_TILES = BB // P  # 8
    N_TILES = OO // 512  # 2

    with tile.TileContext(nc) as tc:
        with (
            tc.tile_pool(name="ops", bufs=1) as opool,
            tc.tile_pool(name="outp", bufs=4) as outpool,
            tc.tile_pool(name="otp", bufs=8) as otpool,
            tc.tile_pool(name="ps", bufs=8, space="PSUM") as pspool,
        ):
            # DMA issue costs ~620ns of sequencer time per dma_start, so spread
            # the issues across four engines' DMA queues to parallelize.
            dma_engines = [nc.sync, nc.scalar]

            fp8 = mm_dtype_name == "float8e4"
            # matmul contraction granularity (2 K-subtiles for fp8 DoubleRow)
            mm_ksub = 2 if fp8 else 1
            # DMA chunk sizes in K-subtiles: a small first chunk lets the first
            # matmul group start early; coarser later chunks amortize the
            # ~600ns per-dma_start sequencer issue cost.
            chunk_sizes = [4, 4] if fp8 else [1] * KT
            assert sum(chunk_sizes) == KT
            chunk_starts = [sum(chunk_sizes[:i]) for i in range(len(chunk_sizes))]
            n_dma = len(chunk_sizes)

            xt_sb = [None] * n_dma
            w_sb = [None] * n_dma
            for k in range(n_dma):
                c0, cs = chunk_starts[k], chunk_sizes[k]
                xtk = opool.tile([P, cs, BB], mm_dt, tag=f"xt{k}")
                nc.sync.dma_start(xtk[:], xt_d[:, c0 : c0 + cs, :])
                wk = opool.tile([P, cs, OO], mm_dt, tag=f"w{k}")
                nc.sync.dma_start(wk[:], w_d[:, c0 : c0 + cs, :])
                xt_sb[k] = xtk
                w_sb[k] = wk
                if k == n_dma - 1:
                    wsq_sb = opool.tile([P, OO], f32, tag="wsq")
                    nc.scalar.dma_start(wsq_sb[:], wsq_d[:])
                    xsq_sb = opool.tile([P, BB // P], f32, tag="xsq")
                    nc.scalar.dma_start(xsq_sb[:], xsq_d[:])

            # map mm index -> (dma chunk, subtile offset within chunk)
            mm_map = []
            for k in range(n_dma):
                for s in range(0, chunk_sizes[k], mm_ksub):
                    mm_map.append((k, s))
            n_mm = len(mm_map)

            # PE warmup: dummy matmuls on a zeroed tile while DMAs stream, so
            # the HAM clock gate opens before the real matmuls begin.
            warm_in = opool.tile([P, 512], mm_dt, tag="warm")
            nc.vector.memset(warm_in[:], 0)
            warm_ps = pspool.tile([P, 512], f32, tag="ps")
            for _ in range(16):
                nc.tensor.matmul(
                    warm_ps[:],
                    lhsT=warm_in[:, :P],
                    rhs=warm_in[:],
                    start=True,
                    stop=True,
                )

            # combined bias tiles on ScalarE (idle during the DMA head):
            # bias_mn[b, o] = xsq[b] + wsq[o]
            bias_sb = []
            for m in range(M_TILES):
                for n in range(N_TILES):
                    bt = outpool.tile([P, 512], f32, tag=f"bias{m}_{n}")
                    nc.scalar.activation(
                        out=bt[:],
                        in_=wsq_sb[:, n * 512 : (n + 1) * 512],
                        func=ident,
                        bias=xsq_sb[:, m : m + 1],
                    )
                    bias_sb.append(bt)

            perf_mode = mybir.MatmulPerfMode.DoubleRow if fp8 else None
            for m in range(M_TILES):
                # full [128, OO] output band staged in SBUF, written by one DMA
                ot = otpool.tile([P, OO], out_dt, tag="ot")
                for n in range(N_TILES):
                    ps = pspool.tile([P, 512], f32, tag="ps")
                    for k in range(n_mm):
                        kd, ki = mm_map[k]
                        if fp8:
                            lhsT = xt_sb[kd][
                                :, ki : ki + 2, m * P : (m + 1) * P
                            ]
                            rhs = w_sb[kd][
                                :, ki : ki + 2, n * 512 : (n + 1) * 512
                            ]
                        else:
                            lhsT = xt_sb[kd][:, ki, m * P : (m + 1) * P]
                            rhs = w_sb[kd][:, ki, n * 512 : (n + 1) * 512]
                        nc.tensor.matmul(
                            ps[:],
                            lhsT=lhsT,
                            rhs=rhs,
                            start=(k == 0),
                            stop=(k == n_mm - 1),
                            perf_mode=perf_mode,
                        )
                    nc.vector.tensor_tensor(
                        ot[:, n * 512 : (n + 1) * 512],
                        ps[:],
                        bias_sb[m * N_TILES + n][:],
                        add,
                    )
                if m == M_TILES - 1:
                    # split the last band across both sequencers so the final
                    # writeback drains in half the time
                    nc.sync.dma_start(
                        out_d[m * P : (m + 1) * P, : OO // 2], ot[:, : OO // 2]
                    )
                    nc.scalar.dma_start(
                        out_d[m * P : (m + 1) * P, OO // 2 :], ot[:, OO // 2 :]
                    )
                else:
                    dma_engines[m % 2].dma_start(out_d[m * P : (m + 1) * P, :], ot[:])
    nc.compile()
    return nc


def _to_mm(a, mm_dtype_name):
    if mm_dtype_name == "bfloat16":
        return a.astype(ml_dtypes.bfloat16)
    if mm_dtype_name == "float8e4":
        return a.astype(ml_dtypes.float8_e4m3)
    if mm_dtype_name == "float32r":
        return _round_fp32r(a)
    return a.astype(np.float32)


def _hi_lo(v, mm_dtype_name):
    """Split fp64 vector into hi+lo parts representable in the mm dtype."""
    hi = _to_mm(v.astype(np.float32), mm_dtype_name)
    lo = _to_mm((v - hi.astype(np.float64)).astype(np.float32), mm_dtype_name)
    return hi, lo


def _shard_inputs(x, W, mm_dtype_name):
    """Build per-core in_maps: augmented x^T and W slices."""
    x = np.asarray(x, dtype=np.float32)
    W = np.asarray(W, dtype=np.float32)
    xsqh = -0.5 * np.einsum("bi,bi->b", x.astype(np.float64), x.astype(np.float64))
    wsqh = -0.5 * np.einsum("io,io->o", W.astype(np.float64), W.astype(np.float64))

    def pi_major(a2d, free):
        """[K, free] -> [P, KT, free] (partition-major)."""
        return np.ascontiguousarray(
            a2d.reshape(KT, P, free).transpose(1, 0, 2)
        )

    xt_shards = []
    xsq_shards = []
    for i in range(R):
        xs = x[i * BB : (i + 1) * BB]
        xt_shards.append(pi_major(_to_mm(np.ascontiguousarray(xs.T), mm_dtype_name), BB))
        xsq_shards.append(
            np.ascontiguousarray(
                xsqh[i * BB : (i + 1) * BB].astype(np.float32).reshape(BB // P, P).T
            )
        )

    w_shards = []
    wsq_shards = []
    for j in range(C):
        w_shards.append(pi_major(_to_mm(W[:, j * OO : (j + 1) * OO], mm_dtype_name), OO))
        wsq_shards.append(
            np.ascontiguousarray(
                np.broadcast_to(
                    wsqh[j * OO : (j + 1) * OO].astype(np.float32), (P, OO)
                )
            )
        )

    in_maps = []
    for core in range(N_CORES):
        i, j = divmod(core, C)
        in_maps.append(
            {
                "xt": xt_shards[i],
                "w": w_shards[j],
                "xsq": xsq_shards[i],
                "wsq": wsq_shards[j],
            }
        )
    return in_maps


def _gather(results):
    out = np.empty((BATCH, D_OUT), dtype=np.float32)
    for core in range(N_CORES):
        i, j = divmod(core, C)
        out[i * BB : (i + 1) * BB, j * OO : (j + 1) * OO] = results[core][
            "out"
        ].astype(np.float32)
    return out


def run(x, W, trace=False, mm_dtype_name="float32r", out16=False):
    from concourse import bass_utils

    key = (mm_dtype_name, out16)
    if key not in _cached:
        _cached[key] = _build_program(mm_dtype_name, out16)
    nc = _cached[key]
    in_maps = _shard_inputs(x, W, mm_dtype_name)
    res = bass_utils.run_bass_kernel_spmd(
        nc, in_maps, core_ids=list(range(N_CORES)), trace=trace
    )
    return _gather(res.results), res


def kernel(x, W):
    out, _ = run(x, W, trace=False, mm_dtype_name="float8e4", out16=True)
    return out


# revision 40
# speedup vs baseline: 1.0361x; 1.0023x over previous
"""Trainium2 Bass kernel for nn_Euclid_FC: out[b,o] = -0.5 * ||x[b,:] - W[:,o]||^2.

Computed as x@W - 0.5*||x_b||^2 - 0.5*||w_o||^2, i.e. a 2048x1024x4096
GEMM plus rank-1 bias terms.

Sharding (8 cores): 2-way over batch x 4-way over the output dim. Each core
computes a [1024, 1024] output block from x^T slice [1024, 1024] and W slice
[1024, 1024] (the traffic-minimal split: 8.5 MiB HBM traffic per core).

Device kernel structure per core:
  - inputs land in SBUF via a few large pi-major-layout DMAs (host
    pre-transposes x and pre-packs [partition, K-subtile, free] order);
  - the GEMM runs as fp8-e4m3 DoubleRow matmuls (2 K-subtiles per matmul,
    ~2x bf16 throughput), accumulating fp32 in PSUM;
  - the rank-1 terms -0.5||x_b||^2 and -0.5||w_o||^2 are precomputed on the
    host (exact fp32), combined into per-tile bias tiles on the otherwise
    idle Scalar engine, and added in the single mandatory PSUM->SBUF pass
    on the Vector engine;
  - dummy warmup matmuls run during the DMA head so the PE clock gate (HAM)
    is open when real matmuls begin;
  - the output is written as fp16 row bands (halves writeback traffic) and
    upcast to fp32 on the host after the gather.

Variants kept for fallback: bfloat16 (rel err ~7e-5) and float32r (fp32 with
11 mantissa bits, rel err ~5e-6) at ~47-65us vs ~34us for fp8.
Measured end-to-end: HW exec ~33.9us, norm rel err ~1.2e-3.
"""

import sys

if "/opt/trn_rl_repo" not in sys.path:
    sys.path.insert(0, "/opt/trn_rl_repo")

import ml_dtypes
import numpy as np

BATCH, D_IN, D_OUT = 2048, 1024, 4096
N_CORES = 8
R, C = 2, 4  # batch split x out-dim split
BB = BATCH // R  # 1024 batch rows per core
OO = D_OUT // C  # 1024 out cols per core
KT = D_IN // 128  # 8 real K-tiles
KA = KT + 1  # +1 augmented bias K-tile
P = 128

_cached = {}


def _round_fp32r(a):
    """Round fp32 array to fp32r (11 explicit mantissa bits), RTNE."""
    b = np.ascontiguousarray(a, dtype=np.float32).view(np.uint32).copy()
    bias = ((b >> 12) & 1) + 0x7FF
    b += bias
    b &= np.uint32(0xFFFFF000)
    return b.view(np.float32)


def _build_program(mm_dtype_name="float32r", out16=False):
    import concourse.mybir as mybir
    import concourse.tile as tile
    from concourse import bacc

    f32 = mybir.dt.float32
    mm_dt = getattr(mybir.dt, mm_dtype_name)

    nc = bacc.Bacc("TRN2", target_bir_lowering=False, debug=False, num_devices=N_CORES)
    # pi-major layout [partition, K-subtile, free], with x^T and W PACKED into
    # one tensor along the free dim so each chunk DMA delivers both matmul
    # operands for those K-subtiles at once (the first group can't start until
    # both are present; packing removes the serial xt-then-w queue latency).
    xw_d = nc.dram_tensor("xw", [P, KT, BB + OO], mm_dt, kind="ExternalInput").ap()
    # wsq replicated across partitions; xsq laid out [b % 128, b // 128]
    wsq_d = nc.dram_tensor("wsq", [P, OO], f32, kind="ExternalInput").ap()
    xsq_d = nc.dram_tensor("xsq", [P, BB // P], f32, kind="ExternalInput").ap()
    out_dt = mybir.dt.float16 if out16 else f32
    out_d = nc.dram_tensor("out", [BB, OO], out_dt, kind="ExternalOutput").ap()

    ident = mybir.ActivationFunctionType.Identity
    add = mybir.AluOpType.add

    M_TILES = BB // P  # 8
    N_TILES = OO // 512  # 2

    with tile.TileContext(nc) as tc:
        with (
            tc.tile_pool(name="ops", bufs=1) as opool,
            tc.tile_pool(name="outp", bufs=4) as outpool,
            tc.tile_pool(name="otp", bufs=8) as otpool,
            tc.tile_pool(name="ps", bufs=8, space="PSUM") as pspool,
        ):
            # DMA issue costs ~620ns of sequencer time per dma_start, so spread
            # the issues across four engines' DMA queues to parallelize.
            dma_engines = [nc.sync, nc.scalar]

            fp8 = mm_dtype_name == "float8e4"
            # matmul contraction granularity (2 K-subtiles for fp8 DoubleRow)
            mm_ksub = 2 if fp8 else 1
            # DMA chunk sizes in K-subtiles: a small first chunk lets the first
            # matmul group start early; coarser later chunks amortize the
            # ~600ns per-dma_start sequencer issue cost.
            chunk_sizes = [2, 2, 4] if fp8 else [1] * KT
            assert sum(chunk_sizes) == KT
            chunk_starts = [sum(chunk_sizes[:i]) for i in range(len(chunk_sizes))]
            n_dma = len(chunk_sizes)

            xw_sb = [None] * n_dma
            for k in range(n_dma):
                c0, cs = chunk_starts[k], chunk_sizes[k]
                xwk = opool.tile([P, cs, BB + OO], mm_dt, tag=f"xw{k}")
                nc.sync.dma_start(xwk[:], xw_d[:, c0 : c0 + cs, :])
                xw_sb[k] = xwk
                if k == n_dma - 1:
                    wsq_sb = opool.tile([P, OO], f32, tag="wsq")
                    nc.scalar.dma_start(wsq_sb[:], wsq_d[:])
                    xsq_sb = opool.tile([P, BB // P], f32, tag="xsq")
                    nc.scalar.dma_start(xsq_sb[:], xsq_d[:])

            # map mm index -> (dma chunk, subtile offset within chunk)
            mm_map = []
            for k in range(n_dma):
                for s in range(0, chunk_sizes[k], mm_ksub):
                    mm_map.append((k, s))
            n_mm = len(mm_map)

            # PE warmup: dummy matmuls on a zeroed tile while DMAs stream, so
            # the HAM clock gate opens before the real matmuls begin.
            warm_in = opool.tile([P, 512], mm_dt, tag="warm")
            nc.vector.memset(warm_in[:], 0)
            warm_ps = pspool.tile([P, 512], f32, tag="ps")
            for _ in range(16):
                nc.tensor.matmul(
                    warm_ps[:],
                    lhsT=warm_in[:, :P],
                    rhs=warm_in[:],
                    start=True,
                    stop=True,
                )

            # combined bias tiles on ScalarE (idle during the DMA head):
            # bias_mn[b, o] = xsq[b] + wsq[o]
            bias_sb = []
            for m in range(M_TILES):
                for n in range(N_TILES):
                    bt = outpool.tile([P, 512], f32, tag=f"bias{m}_{n}")
                    nc.scalar.activation(
                        out=bt[:],
                        in_=wsq_sb[:, n * 512 : (n + 1) * 512],
                        func=ident,
                        bias=xsq_sb[:, m : m + 1],
                    )
                    bias_sb.append(bt)

            perf_mode = mybir.MatmulPerfMode.DoubleRow if fp8 else None
            for m in range(M_TILES):
                # full [128, OO] output band staged in SBUF, written by one DMA
                ot = otpool.tile([P, OO], out_dt, tag="ot")
                for n in range(N_TILES):
                    ps = pspool.tile([P, 512], f32, tag="ps")
                    for k in range(n_mm):
                        kd, ki = mm_map[k]
                        if fp8:
                            lhsT = xw_sb[kd][
                                :, ki : ki + 2, m * P : (m + 1) * P
                            ]
                            rhs = xw_sb[kd][
                                :, ki : ki + 2, BB + n * 512 : BB + (n + 1) * 512
                            ]
                        else:
                            lhsT = xw_sb[kd][:, ki, m * P : (m + 1) * P]
                            rhs = xw_sb[kd][:, ki, BB + n * 512 : BB + (n + 1) * 512]
                        nc.tensor.matmul(
                            ps[:],
                            lhsT=lhsT,
                            rhs=rhs,
                            start=(k == 0),
                            stop=(k == n_mm - 1),
                            perf_mode=perf_mode,
                        )
                    nc.vector.tensor_tensor(
                        ot[:, n * 512 : (n + 1) * 512],
                        ps[:],
                        bias_sb[m * N_TILES + n][:],
                        add,
                    )
                if m == M_TILES - 1:
                    # split the last band across both sequencers so the final
                    # writeback drains in half the time
                    nc.sync.dma_start(
                        out_d[m * P : (m + 1) * P, : OO // 2], ot[:, : OO // 2]
                    )
                    nc.scalar.dma_start(
                        out_d[m * P : (m + 1) * P, OO // 2 :], ot[:, OO // 2 :]
                    )
                else:
                    dma_engines[m % 2].dma_start(out_d[m * P : (m + 1) * P, :], ot[:])
    nc.compile()
    return nc


def _to_mm(a, mm_dtype_name):
    if mm_dtype_name == "bfloat16":
        return a.astype(ml_dtypes.bfloat16)
    if mm_dtype_name == "float8e4":
        return a.astype(ml_dtypes.float8_e4m3)
    if mm_dtype_name == "float32r":
        return _round_fp32r(a)
    return a.astype(np.float32)


def _hi_lo(v, mm_dtype_name):
    """Split fp64 vector into hi+lo parts representable in the mm dtype."""
    hi = _to_mm(v.astype(np.float32), mm_dtype_name)
    lo = _to_mm((v - hi.astype(np.float64)).astype(np.float32), mm_dtype_name)
    return hi, lo


def _shard_inputs(x, W, mm_dtype_name):
    """Build per-core in_maps: augmented x^T and W slices."""
    x = np.asarray(x, dtype=np.float32)
    W = np.asarray(W, dtype=np.float32)
    xsqh = -0.5 * np.einsum("bi,bi->b", x.astype(np.float64), x.astype(np.float64))
    wsqh = -0.5 * np.einsum("io,io->o", W.astype(np.float64), W.astype(np.float64))

    def pi_major(a2d, free):
        """[K, free] -> [P, KT, free] (partition-major)."""
        return np.ascontiguousarray(
            a2d.reshape(KT, P, free).transpose(1, 0, 2)
        )

    xt_shards = []
    xsq_shards = []
    for i in range(R):
        xs = x[i * BB : (i + 1) * BB]
        xt_shards.append(pi_major(_to_mm(np.ascontiguousarray(xs.T), mm_dtype_name), BB))
        xsq_shards.append(
            np.ascontiguousarray(
                xsqh[i * BB : (i + 1) * BB].astype(np.float32).reshape(BB // P, P).T
            )
        )

    w_shards = []
    wsq_shards = []
    for j in range(C):
        w_shards.append(pi_major(_to_mm(W[:, j * OO : (j + 1) * OO], mm_dtype_name), OO))
        wsq_shards.append(
            np.ascontiguousarray(
                np.broadcast_to(
                    wsqh[j * OO : (j + 1) * OO].astype(np.float32), (P, OO)
                )
            )
        )

    # pack x^T and W along the free dim: [P, KT, BB + OO]
    xw_shards = {}
    for core in range(N_CORES):
        i, j = divmod(core, C)
        if (i, j) not in xw_shards:
            xw_shards[(i, j)] = np.ascontiguousarray(
                np.concatenate([xt_shards[i], w_shards[j]], axis=2)
            )

    in_maps = []
    for core in range(N_CORES):
        i, j = divmod(core, C)
        in_maps.append(
            {
                "xw": xw_shards[(i, j)],
                "xsq": xsq_shards[i],
                "wsq": wsq_shards[j],
            }
        )
    return in_maps


def _gather(results):
    out = np.empty((BATCH, D_OUT), dtype=np.float32)
    for core in range(N_CORES):
        i, j = divmod(core, C)
        out[i * BB : (i + 1) * BB, j * OO : (j + 1) * OO] = results[core][
            "out"
        ].astype(np.float32)
    return out


def run(x, W, trace=False, mm_dtype_name="float32r", out16=False):
    from concourse import bass_utils

    key = (mm_dtype_name, out16)
    if key not in _cached:
        _cached[key] = _build_program(mm_dtype_name, out16)
    nc = _cached[key]
    in_maps = _shard_inputs(x, W, mm_dtype_name)
    res = bass_utils.run_bass_kernel_spmd(
        nc, in_maps, core_ids=list(range(N_CORES)), trace=trace
    )
    return _gather(res.results), res


def kernel(x, W):
    out, _ = run(x, W, trace=False, mm_dtype_name="float8e4", out16=True)
    return out
